# revision 5
# baseline (speedup 1.0000x reference)
"""Trainium2 Bass kernel: MultiHeadLatentAttention (bf16 pipeline).

Problem (hardcoded): B=4, S=1024, HID=2048, NH=16 heads of HD=128, LAT=512,
fp32 in/out, causal attention with RoPE, latent-compressed K/V (MLA).

Sharding over 8 NeuronCores: core c = (batch b = c//2, head-group hg = c%2).
Each core handles one batch element and 8 heads (local width HL=1024).

All matmul operands are bf16 (host casts); PSUM accumulation is fp32.

Device layout (contraction dim always on SBUF partitions; all SBUF tiles
flat 2D [128, cols]):
  xT   [P, 16*S] bf16 (host pre-swizzled x[b].T)
  QT = (x Wq + bq).T -> qT [P, 8*S];  latT = (x Wdown).T -> [P, 4*S]
  KT = (lat Wk_up).T -> kT [P, 8*S];  V natural -> v [P, 8*HL]
  RoPE per head-pair on [P, 2S] tiles: out = raw*cos2 + shift64(raw)*sin2e;
    the partition shift is two SBUF->SBUF DMAs on the SCALAR queue.
  scoresT_h = k_h @ q_h.T in [k,q] blocks; diagonal blocks column-sliced,
    residual triangle zeroed by a tri mask.
  ex = exp(scores/sqrt(128)) bf16
  sums: ONE PSUM tile [8,512] per q-chunk accumulates sel-ones matmuls for
    all 8 heads -> one reciprocal serves the whole pass.
  ctxT unnormalized bf16; normalized via bc = sel8^T @ rec broadcast matmul.

Phase C is two passes: pass Q0 = all 8 heads' q-chunk 0 (two heads in
flight); pass Q1 = all 8 heads' q-chunk 1 with the ENTIRE q-chunk-0
out-projection interleaved as PE filler between attention dependency
steps.  The q-chunk-1 out-projection runs dense at the end.

Phase-A DMA is issued in PE consumption order (x/wq chunks alternating,
first pair small) so the PE never starves during ramp-in.  V up-proj
uses 4-bank PSUM groups so attention's banks drain early.

Host gathers: out[b] = (outT[2b] + outT[2b+1]).T + bo.
"""

import os

if "axon" not in os.environ.get("JAX_PLATFORMS", ""):
    os.environ["JAX_PLATFORMS"] = "axon"

import contextlib

import ml_dtypes
import numpy as np

import concourse.bacc as bacc
import concourse.mybir as mybir
import concourse.tile as tile
from concourse.bass_utils import run_bass_kernel_spmd

# ---- problem dims (hardcoded per contest rules)
B, S, HID, NH, LAT = 4, 1024, 2048, 16, 512
HD = 128
NHL = NH // 2          # heads per core = 8
HL = NHL * HD          # local head width = 1024
P = 128
KT_H = HID // P        # 16
KT_L = LAT // P        # 4
QCW = 512              # q-chunk width (PSUM bank = 512 fp32)
NQC = S // QCW         # 2
SC_SCALE = float(1.0 / np.sqrt(HD))

F32 = mybir.dt.float32
BF16 = mybir.dt.bfloat16
NPBF = ml_dtypes.bfloat16

N_CORES = 8
CPACK_W = 2 * S + 2 * S + P + 8 * P   # cos2 | sin2 | tri | selones8


def build_bass(loop_iters=None):
    nc = bacc.Bacc("TRN2", target_bir_lowering=False, debug=False, num_devices=8)

    xTd = nc.dram_tensor("xT", [P, KT_H, S], BF16, kind="ExternalInput")[:]
    wqd = nc.dram_tensor("wq", [P, KT_H, HL], BF16, kind="ExternalInput")[:]
    wdownd = nc.dram_tensor("wdown", [P, KT_H, LAT], BF16, kind="ExternalInput")[:]
    wkupd = nc.dram_tensor("wkup", [P, KT_L, HL], BF16, kind="ExternalInput")[:]
    wvupd = nc.dram_tensor("wvup", [P, KT_L, HL], BF16, kind="ExternalInput")[:]
    wod = nc.dram_tensor("wo", [P, NHL, HID], BF16, kind="ExternalInput")[:]
    bqd = nc.dram_tensor("bq", [P, NHL], F32, kind="ExternalInput")[:]
    cpackd = nc.dram_tensor("cpack", [P, CPACK_W], BF16, kind="ExternalInput")[:]
    sel8d = nc.dram_tensor("sel8", [NHL, NHL * P], BF16, kind="ExternalInput")[:]
    outTd = nc.dram_tensor("outT", [HID, S], BF16, kind="ExternalOutput")[:]

    with tile.TileContext(nc) as tc, contextlib.ExitStack() as _les:
        if loop_iters is not None:
            _les.enter_context(tc.For_i(0, loop_iters, 1))
        with (
            tc.tile_pool(name="consts", bufs=1) as consts,
            tc.tile_pool(name="resident", bufs=1) as resident,
        ):
            cpack = consts.tile([P, CPACK_W], BF16)
            cos2_sb = cpack[:, 0:2 * S]
            sin2_sb = cpack[:, 2 * S:4 * S]
            tri_sb = cpack[:, 4 * S:4 * S + P]
            selo_sb = cpack[:, 4 * S + P:4 * S + P + NHL * P]
            bq_sb = consts.tile([P, NHL], F32)
            sel8_sb = consts.tile([NHL, NHL * P], BF16)

            latT = resident.tile([P, KT_L * S], BF16)
            qT = resident.tile([P, NHL * S], BF16)
            kT = resident.tile([P, NHL * S], BF16)
            v_sb = resident.tile([P, NHL * HL], BF16)
            ctxT = resident.tile([P, NHL * S], BF16)
            # phase-B weights, loaded during phase A
            wkg = resident.tile([P, KT_L * HL], BF16)
            wvg = resident.tile([P, KT_L * HL], BF16)

            pacc_cm = tc.tile_pool(name="pacc", bufs=8, space="PSUM")
            pacc = pacc_cm.__enter__()

            def rope_pair(rp, h, ps4, dst, bias, sin_eng,
                          add_eng=None, dma_eng=None):
                """RoPE for heads h, h+1 from 4 psum tiles [(j,ntc)]."""
                add_eng = add_eng or nc.vector
                dma_eng = dma_eng or nc.scalar
                raw = rp.tile([P, 2 * S], BF16, tag="raw", name="raw")
                sh = rp.tile([P, 2 * S], BF16, tag="sh", name="sh")
                for j in range(2):
                    for ntc in range(NQC):
                        seg = raw[:, (j * NQC + ntc) * QCW:
                                  (j * NQC + ntc + 1) * QCW]
                        if bias:
                            nc.scalar.add(seg, ps4[j * 2 + ntc],
                                          bq_sb[:, h + j:h + j + 1])
                        else:
                            nc.scalar.copy(seg, ps4[j * 2 + ntc])
                    # per-head shift: unblocks as soon as this head's two
                    # segment copies land (not the whole pair)
                    dma_eng.dma_start(sh[0:64, j * S:(j + 1) * S],
                                      raw[64:128, j * S:(j + 1) * S])
                    dma_eng.dma_start(sh[64:128, j * S:(j + 1) * S],
                                      raw[0:64, j * S:(j + 1) * S])
                out = dst[:, h * S:(h + 2) * S]
                nc.vector.tensor_mul(out, raw, cos2_sb)
                sin_eng.tensor_mul(sh, sh, sin2_sb)
                add_eng.tensor_add(out, out, sh)

            # ---------- phase A: QT (2 groups of 4 heads) + latT ----------
            with (
                tc.tile_pool(name="xp", bufs=1) as xp,
                tc.tile_pool(name="ws1", bufs=2) as ws1,
                tc.tile_pool(name="ropeA", bufs=2) as rpA,
            ):
                xT_sb = xp.tile([P, KT_H * S], BF16)
                wg0 = ws1.tile([P, KT_H * QCW], BF16, tag="w", name="wg")
                # consts stream on the SCALAR queue: doesn't head-of-line
                # block the x/w stream, needed only after og0 matmuls
                nc.scalar.dma_start(cpack, cpackd)
                nc.scalar.dma_start(sel8_sb, sel8d)
                nc.scalar.dma_start(bq_sb, bqd)
                # x/wq interleaved in PE consumption order (kt ascending);
                # first pair small so the PE starts ~1us in
                nc.sync.dma_start(xT_sb[:, 0:S], xTd[:, 0:1, :])
                nc.sync.dma_start(wg0[:, 0:2 * QCW], wqd[:, 0:2, 0:QCW])
                nc.sync.dma_start(xT_sb[:, S:2 * S], xTd[:, 1:2, :])
                nc.sync.dma_start(wg0[:, 2 * QCW:4 * QCW],
                                  wqd[:, 2:4, 0:QCW])
                nc.sync.dma_start(xT_sb[:, 2 * S:4 * S], xTd[:, 2:4, :])
                nc.sync.dma_start(wg0[:, 4 * QCW:8 * QCW],
                                  wqd[:, 4:8, 0:QCW])
                nc.sync.dma_start(xT_sb[:, 4 * S:6 * S], xTd[:, 4:6, :])
                nc.sync.dma_start(wg0[:, 8 * QCW:12 * QCW],
                                  wqd[:, 8:12, 0:QCW])
                nc.sync.dma_start(xT_sb[:, 6 * S:8 * S], xTd[:, 6:8, :])
                nc.sync.dma_start(wg0[:, 12 * QCW:16 * QCW],
                                  wqd[:, 12:16, 0:QCW])
                nc.sync.dma_start(xT_sb[:, 8 * S:12 * S], xTd[:, 8:12, :])
                nc.sync.dma_start(xT_sb[:, 12 * S:16 * S], xTd[:, 12:16, :])
                # preload the exp table set while the PE is busy with
                # projections (first ACTIVATE otherwise pays ~2.7us in C)
                warm = ws1.tile([1, NHL], F32, tag="warm", name="warm")
                nc.scalar.activation(
                    out=warm, in_=bq_sb[0:1, :],
                    func=mybir.ActivationFunctionType.Exp, scale=1.0)

                for og in range(2):
                    if og == 0:
                        wg = wg0
                    else:
                        wg = ws1.tile([P, KT_H * QCW], BF16, tag="w",
                                      name="wg")
                        for hf in range(2):   # two 1MB halves
                            nc.sync.dma_start(
                                wg[:, hf * 8 * QCW:(hf + 1) * 8 * QCW],
                                wqd[:, hf * 8:(hf + 1) * 8,
                                    og * QCW:(og + 1) * QCW])
                    ps = [pacc.tile([P, QCW], F32, tag="acc", name="acc")
                          for _ in range(8)]
                    for kt in range(KT_H):
                        for oi in range(4):
                            for ntc in range(NQC):
                                nc.tensor.matmul(
                                    ps[oi * 2 + ntc],
                                    lhsT=wg[:, kt * QCW + oi * P:
                                            kt * QCW + (oi + 1) * P],
                                    rhs=xT_sb[:, kt * S + ntc * QCW:
                                              kt * S + (ntc + 1) * QCW],
                                    start=(kt == 0),
                                    stop=(kt == KT_H - 1),
                                )
                    for pr in range(2):
                        rope_pair(rpA, og * 4 + pr * 2,
                                  ps[pr * 4:pr * 4 + 4], qT, bias=True,
                                  sin_eng=nc.gpsimd)

                # latT group (4 out tiles x 2 chunks)
                wg = ws1.tile([P, KT_H * QCW], BF16, tag="w", name="wg")
                for hf in range(2):
                    nc.sync.dma_start(
                        wg[:, hf * 8 * QCW:(hf + 1) * 8 * QCW],
                        wdownd[:, hf * 8:(hf + 1) * 8, :])
                nc.sync.dma_start(wkg, wkupd)
                nc.sync.dma_start(wvg, wvupd)
                ps = [pacc.tile([P, QCW], F32, tag="acc", name="acc")
                      for _ in range(8)]
                for kt in range(KT_H):
                    for oi in range(4):
                        for ntc in range(NQC):
                            nc.tensor.matmul(
                                ps[oi * 2 + ntc],
                                lhsT=wg[:, kt * QCW + oi * P:
                                        kt * QCW + (oi + 1) * P],
                                rhs=xT_sb[:, kt * S + ntc * QCW:
                                          kt * S + (ntc + 1) * QCW],
                                start=(kt == 0),
                                stop=(kt == KT_H - 1),
                            )
                for oi in range(4):
                    for ntc in range(NQC):
                        dstap = latT[:, oi * S + ntc * QCW:
                                     oi * S + (ntc + 1) * QCW]
                        if ntc == 0:
                            nc.scalar.copy(dstap, ps[oi * 2 + ntc])
                        else:
                            nc.vector.tensor_copy(dstap, ps[oi * 2 + ntc])

            # ---------- phase B: KT (rope) then V ----------
            wop_cm = tc.tile_pool(name="wop", bufs=1)
            wop = wop_cm.__enter__()
            wo_sb = wop.tile([P, NHL * HID], BF16)
            nc.sync.dma_start(wo_sb, wod)

            rpB_cm = tc.tile_pool(name="ropeB", bufs=2)
            rpB = rpB_cm.__enter__()

            for og in range(2):
                ps = [pacc.tile([P, QCW], F32, tag="acc", name="acc")
                      for _ in range(8)]
                for kt in range(KT_L):
                    for oi in range(4):
                        for ntc in range(NQC):
                            nc.tensor.matmul(
                                ps[oi * 2 + ntc],
                                lhsT=wkg[:, kt * HL + og * 4 * P + oi * P:
                                         kt * HL + og * 4 * P
                                         + (oi + 1) * P],
                                rhs=latT[:, kt * S + ntc * QCW:
                                         kt * S + (ntc + 1) * QCW],
                                start=(kt == 0),
                                stop=(kt == KT_L - 1),
                            )
                for pr in range(2):
                    sin = nc.vector if og == 0 else nc.gpsimd
                    rope_pair(rpB, og * 4 + pr * 2,
                              ps[pr * 4:pr * 4 + 4], kT, bias=False,
                              sin_eng=sin, add_eng=nc.vector,
                              dma_eng=nc.sync)

            # V up-proj in 4-bank groups so the last groups' copies drain
            # while later matmuls run and attention's PSUM banks free early
            for hlc in range(2):
                for g in range(2):
                    ps = [pacc.tile([P, QCW], F32, tag="acc", name="acc")
                          for _ in range(4)]
                    for kt in range(KT_L):
                        for s4 in range(4):
                            st = g * 4 + s4
                            nc.tensor.matmul(
                                ps[s4],
                                lhsT=latT[:, kt * S + st * P:
                                          kt * S + (st + 1) * P],
                                rhs=wvg[:, kt * HL + hlc * QCW:
                                        kt * HL + (hlc + 1) * QCW],
                                start=(kt == 0),
                                stop=(kt == KT_L - 1),
                            )
                    for s4 in range(4):
                        st = g * 4 + s4
                        dstap = v_sb[:, st * HL + hlc * QCW:
                                     st * HL + (hlc + 1) * QCW]
                        if s4 % 2 == 1:
                            nc.vector.tensor_copy(dstap, ps[s4])
                        else:
                            nc.scalar.copy(dstap, ps[s4])

            rpB_cm.__exit__(None, None, None)
            pacc_cm.__exit__(None, None, None)

            # ---------- phase C: attention + out-projection ----------
            with (
                tc.tile_pool(name="pctx", bufs=2, space="PSUM") as pctx,
                tc.tile_pool(name="psums", bufs=2, space="PSUM") as psums,
                tc.tile_pool(name="exla", bufs=3) as exla,
                tc.tile_pool(name="exlb", bufs=3) as exlb,
                tc.tile_pool(name="small", bufs=2) as small,
                tc.tile_pool(name="outsb", bufs=3) as outsb,
            ):
                fill = []

                def fl_pop(n, reserve=0):
                    for _ in range(n):
                        if len(fill) > reserve:
                            fill.pop(0)()

                def add_outproj(qc, ot, pbc, store_eng):
                    st_ = {}

                    def mk_mm(kt):
                        def go():
                            if kt == 0:
                                st_["po"] = pbc.tile([P, QCW], F32,
                                                     tag="bcpo", name="po")
                            nc.tensor.matmul(
                                st_["po"],
                                lhsT=wo_sb[:, kt * HID + ot * P:
                                           kt * HID + (ot + 1) * P],
                                rhs=ctxT[:, kt * S + qc * QCW:
                                         kt * S + (qc + 1) * QCW],
                                start=(kt == 0),
                                stop=(kt == NHL - 1),
                            )
                        return go

                    def fin():
                        ob = outsb.tile([P, QCW], BF16, tag="osb", name="ob")
                        nc.vector.tensor_copy(ob, st_["po"])
                        store_eng.dma_start(
                            outTd[ot * P:(ot + 1) * P,
                                  qc * QCW:(qc + 1) * QCW], ob)

                    for kt in range(NHL):
                        fill.append(mk_mm(kt))
                    fill.append(fin)

                def att_unit(h, qc, sums_ps, sctr, stot, exp_pool, psc,
                             tri_engs):
                    """Generator: one attention head, yields per kt step."""
                    nkt = 4 * qc + 4
                    ctx = pctx.tile([P, QCW], F32, tag="ctx", name="ctx")

                    def geom(kt):
                        off = kt - 4 * qc
                        if off < 0:
                            return 0, QCW, False
                        return 128 * off, QCW - 128 * off, True

                    def emit_sc(kt):
                        lo, w, diag = geom(kt)
                        sc = psc.tile([P, QCW], F32, tag="sc", name="sc")
                        nc.tensor.matmul(
                            sc[:, :w],
                            lhsT=kT[:, h * S + kt * P:h * S + (kt + 1) * P],
                            rhs=qT[:, h * S + qc * QCW + lo:
                                   h * S + qc * QCW + lo + w],
                            start=True, stop=True,
                        )
                        ex = exp_pool.tile([P, QCW], BF16, tag="ex",
                                           name="ex")
                        nc.scalar.activation(
                            out=ex[:, :w], in_=sc[:, :w],
                            func=mybir.ActivationFunctionType.Exp,
                            scale=SC_SCALE,
                        )
                        if diag:
                            eng = tri_engs[kt % 2]
                            eng.tensor_mul(ex[:, 0:P], ex[:, 0:P], tri_sb)
                        return ex

                    def emit_pv(kt, ex):
                        lo, w, _ = geom(kt)
                        nc.tensor.matmul(
                            ctx[:, lo:lo + w],
                            lhsT=v_sb[:, kt * HL + h * P:
                                      kt * HL + (h + 1) * P],
                            rhs=ex[:, :w],
                            start=(kt == 0),
                            stop=(kt == nkt - 1),
                        )
                        i = sctr[0]
                        sctr[0] += 1
                        nc.tensor.matmul(
                            sums_ps[:, lo:lo + w],
                            lhsT=selo_sb[:, h * P:(h + 1) * P],
                            rhs=ex[:, :w],
                            start=(i == 0),
                            stop=(i == stot - 1),
                        )

                    exs = {0: emit_sc(0)}
                    for kt in range(nkt):
                        if kt + 1 < nkt:
                            exs[kt + 1] = emit_sc(kt + 1)
                        emit_pv(kt, exs.pop(kt))
                        yield
                    ctx_dst = ctxT[:, h * S + qc * QCW:
                                   h * S + (qc + 1) * QCW]
                    if qc == 0:
                        if h % 2 == 0:
                            nc.scalar.copy(ctx_dst, ctx)
                        else:
                            nc.vector.tensor_copy(ctx_dst, ctx)
                    else:
                        nc.vector.tensor_copy(ctx_dst, ctx)

                def drive(gens, live_n, pops, reserve=0):
                    """Rolling scheduler: keep live_n generators in flight;
                    pop `pops` fill closures after each yield, keeping
                    `reserve` closures back for the pass boundary."""
                    q = list(gens)
                    live = []
                    while q or live:
                        while q and len(live) < live_n:
                            live.append(q.pop(0))
                        for g in list(live):
                            try:
                                next(g)
                            except StopIteration:
                                live.remove(g)
                            fl_pop(pops, reserve)

                def finish8(qc, sums_ps, pbc_ref):
                    """Copy sums rows out, reciprocal, return norm closure."""
                    srow = small.tile([NHL, QCW], F32, tag="srow",
                                      name="srow")
                    nc.vector.tensor_copy(srow, sums_ps[0:NHL, :])
                    rec = small.tile([NHL, QCW], BF16, tag="rec", name="rec")
                    with nc.allow_low_precision(reason="bf16 softmax "
                                                "denominator (gate 2e-2)"):
                        nc.vector.reciprocal(out=rec, in_=srow)

                    def go():
                        for h in range(NHL):
                            bc = pbc_ref[0].tile([P, QCW], F32, tag="bcpo",
                                                 name="bc")
                            nc.tensor.matmul(
                                bc,
                                lhsT=sel8_sb[:, h * P:(h + 1) * P],
                                rhs=rec,
                                start=True, stop=True,
                            )
                            sl = ctxT[:, h * S + qc * QCW:
                                      h * S + (qc + 1) * QCW]
                            nc.vector.tensor_mul(sl, sl, bc)
                    return go

                pbc_ref = [None]

                # ---- pass Q0: all 8 heads, q-chunk 0 (two heads in flight)
                pscq0_cm = tc.tile_pool(name="pscq0", bufs=4, space="PSUM")
                pscq0 = pscq0_cm.__enter__()
                sums_q0 = psums.tile([P, QCW], F32, tag="sums", name="sums")
                c0 = [0]
                gens = [
                    att_unit(h, 0, sums_q0, c0, NHL * 4,
                             exla if h % 2 == 0 else exlb, pscq0,
                             (nc.vector, nc.gpsimd) if h % 2 == 0
                             else (nc.gpsimd, nc.vector))
                    for h in range(NHL)
                ]
                drive(gens, live_n=2, pops=1)
                norm_q0 = finish8(0, sums_q0, pbc_ref)
                pscq0_cm.__exit__(None, None, None)

                # ---- pass Q1: all 8 heads q-chunk 1; fill = qc0 norms +
                # qc0 out-projection
                with (
                    tc.tile_pool(name="pscq1", bufs=2, space="PSUM") as pscq1,
                    tc.tile_pool(name="pbc", bufs=2, space="PSUM") as pbc,
                ):
                    pbc_ref[0] = pbc
                    fill.append(norm_q0)
                    for ot in range(HID // P):
                        add_outproj(0, ot, pbc, nc.sync)
                    sums_q1 = psums.tile([P, QCW], F32, tag="sums",
                                         name="sums")
                    c1 = [0]
                    gens = [
                        att_unit(h, 1, sums_q1, c1, NHL * 8,
                                 exla if h % 2 == 0 else exlb, pscq1,
                                 (nc.gpsimd, nc.vector))
                        for h in range(NHL)
                    ]
                    drive(gens, live_n=1, pops=2, reserve=18)
                    # issue sums copy + reciprocal now; drain leftover qc0
                    # out-proj to cover the reciprocal latency
                    norm_q1 = finish8(1, sums_q1, pbc_ref)
                    while fill:
                        fill.pop(0)()
                    norm_q1()
                    for ot in range(HID // P):
                        add_outproj(1, ot, pbc,
                                    nc.scalar if ot >= 10 else nc.sync)
                    while fill:
                        fill.pop(0)()

            wop_cm.__exit__(None, None, None)
    nc.compile()
    return nc


# ---------------- host side ----------------

def _host_consts():
    inv_freq = 1.0 / (10000.0 ** (np.arange(0, HD, 2, dtype=np.float64) / HD))
    t = np.arange(S, dtype=np.float64)
    freqs = t[:, None] * inv_freq[None, :]            # [S, 64]
    emb = np.concatenate([freqs, freqs], axis=-1)     # [S, 128]
    cosT = np.cos(emb).T.astype(np.float32)           # [128, S]
    sinT = np.sin(emb).T.astype(np.float32)
    sinTe = sinT.copy()
    sinTe[:64] *= -1.0                                # rotate_half sign folded
    cos2 = np.broadcast_to(cosT[:, None, :], (P, 2, S)).reshape(P, 2 * S)
    sin2 = np.broadcast_to(sinTe[:, None, :], (P, 2, S)).reshape(P, 2 * S)

    ii = np.arange(P)[:, None]
    tri = (np.arange(P)[None, :] - ii >= 0).astype(np.float32)  # [128,128]

    selones = np.zeros((P, NHL * P), dtype=np.float32)
    for h in range(NHL):
        selones[:, h * P + h] = 1.0
    cpack = np.ascontiguousarray(
        np.concatenate([cos2, sin2, tri, selones], axis=1)).astype(NPBF)

    sel8 = np.zeros((NHL, NHL * P), dtype=NPBF)
    for h in range(NHL):
        sel8[h, h * P:(h + 1) * P] = 1.0
    return cpack, sel8


_CACHE = {}


def _get_built():
    if "nc" not in _CACHE:
        _CACHE["nc"] = build_bass()
        _CACHE["consts"] = _host_consts()
    return _CACHE["nc"], _CACHE["consts"]


def _swz(a, n_kt):
    """[n_kt*128, W] -> [128, n_kt, W] (partition-major swizzle), bf16."""
    w = a.shape[1]
    return np.ascontiguousarray(
        a.reshape(n_kt, P, w).transpose(1, 0, 2)).astype(NPBF)


def make_in_maps(x, Wq, bq, Wdown, Wk_up, Wv_up, Wo):
    cpack, sel8 = _get_built()[1]
    in_maps = []
    for c in range(N_CORES):
        b, hg = c // 2, c % 2
        sl = slice(hg * HL, (hg + 1) * HL)
        in_maps.append({
            "xT": _swz(np.ascontiguousarray(x[b].T), KT_H),
            "wq": _swz(Wq[:, sl], KT_H),
            "wdown": _swz(Wdown, KT_H),
            "wkup": _swz(Wk_up[:, sl], KT_L),
            "wvup": _swz(Wv_up[:, sl], KT_L),
            "wo": _swz(Wo[sl, :], NHL),
            "bq": np.ascontiguousarray(
                bq[sl].reshape(NHL, P).T).astype(np.float32),
            "cpack": cpack,
            "sel8": sel8,
        })
    return in_maps


def gather_out(results, bo):
    out = np.empty((B, S, HID), dtype=np.float32)
    for b in range(B):
        acc = (results[2 * b]["outT"].astype(np.float32)
               + results[2 * b + 1]["outT"].astype(np.float32))  # [HID, S]
        out[b] = acc.T + bo[None, :]
    return out


def kernel(x, Wq, bq, Wdown, Wk_up, Wv_up, Wo, bo):
    x = np.asarray(x, dtype=np.float32)
    Wq = np.asarray(Wq, dtype=np.float32)
    bq = np.asarray(bq, dtype=np.float32)
    Wdown = np.asarray(Wdown, dtype=np.float32)
    Wk_up = np.asarray(Wk_up, dtype=np.float32)
    Wv_up = np.asarray(Wv_up, dtype=np.float32)
    Wo = np.asarray(Wo, dtype=np.float32)
    bo = np.asarray(bo, dtype=np.float32)

    nc, _ = _get_built()
    in_maps = make_in_maps(x, Wq, bq, Wdown, Wk_up, Wv_up, Wo)
    res = run_bass_kernel_spmd(nc, in_maps, core_ids=list(range(N_CORES)))
    return gather_out(res.results, bo)


# revision 13
# speedup vs baseline: 1.0574x; 1.0574x over previous
"""Trainium2 Bass kernel: MultiHeadLatentAttention (bf16 pipeline).

Problem (hardcoded): B=4, S=1024, HID=2048, NH=16 heads of HD=128, LAT=512,
fp32 in/out, causal attention with RoPE, latent-compressed K/V (MLA).

Sharding over 8 NeuronCores: core c = (batch b = c//2, head-group hg = c%2).
Each core handles one batch element and 8 heads (local width HL=1024).

All matmul operands are bf16 (host casts); PSUM accumulation is fp32.

Device layout (contraction dim always on SBUF partitions; all SBUF tiles
flat 2D [128, cols]):
  xT   [P, 16*S] bf16 (host pre-swizzled x[b].T)
  QT = (x Wq + bq).T -> qT [P, 8*S];  latT = (x Wdown).T -> [P, 4*S]
  KT = (lat Wk_up).T -> kT [P, 8*S];  V natural -> v [P, 8*HL]
  RoPE per head-pair on [P, 2S] tiles: out = raw*cos2 + shift64(raw)*sin2e;
    the partition shift is two SBUF->SBUF DMAs on the SCALAR queue.
  scoresT_h = k_h @ q_h.T in [k,q] blocks; diagonal blocks column-sliced,
    residual triangle zeroed by a tri mask.
  ex = exp(scores/sqrt(128)) bf16
  sums: ONE PSUM tile [8,512] per q-chunk accumulates sel-ones matmuls for
    all 8 heads -> one reciprocal serves the whole pass.
  ctxT unnormalized bf16; normalized via bc = sel8^T @ rec broadcast matmul.

Phase C is two passes: pass Q0 = all 8 heads' q-chunk 0 (two heads in
flight); pass Q1 = all 8 heads' q-chunk 1 with the ENTIRE q-chunk-0
out-projection interleaved as PE filler between attention dependency
steps.  The q-chunk-1 out-projection runs dense at the end.

Phase-A DMA is issued in PE consumption order (x/wq chunks alternating,
first pair small) so the PE never starves during ramp-in.  V up-proj
uses 4-bank PSUM groups so attention's banks drain early.

Host gathers: out[b] = (outT[2b] + outT[2b+1]).T + bo.
"""

import os

if "axon" not in os.environ.get("JAX_PLATFORMS", ""):
    os.environ["JAX_PLATFORMS"] = "axon"

import contextlib

import ml_dtypes
import numpy as np

import concourse.bacc as bacc
import concourse.mybir as mybir
import concourse.tile as tile
from concourse.bass_utils import run_bass_kernel_spmd

# ---- problem dims (hardcoded per contest rules)
B, S, HID, NH, LAT = 4, 1024, 2048, 16, 512
HD = 128
NHL = NH // 2          # heads per core = 8
HL = NHL * HD          # local head width = 1024
P = 128
KT_H = HID // P        # 16
KT_L = LAT // P        # 4
QCW = 512              # q-chunk width (PSUM bank = 512 fp32)
NQC = S // QCW         # 2
SC_SCALE = float(1.0 / np.sqrt(HD))

F32 = mybir.dt.float32
BF16 = mybir.dt.bfloat16
NPBF = ml_dtypes.bfloat16

N_CORES = 8
CPACK_W = 2 * S + 2 * S + P + 8 * P   # cos2 | sin2 | tri | selones8


def build_bass(loop_iters=None):
    nc = bacc.Bacc("TRN2", target_bir_lowering=False, debug=False, num_devices=8)

    xTd = nc.dram_tensor("xT", [P, KT_H, S], BF16, kind="ExternalInput")[:]
    wqd = nc.dram_tensor("wq", [P, KT_H, HL], BF16, kind="ExternalInput")[:]
    wdownd = nc.dram_tensor("wdown", [P, KT_H, LAT], BF16, kind="ExternalInput")[:]
    wkupd = nc.dram_tensor("wkup", [P, KT_L, HL], BF16, kind="ExternalInput")[:]
    wvupd = nc.dram_tensor("wvup", [P, KT_L, HL], BF16, kind="ExternalInput")[:]
    wod = nc.dram_tensor("wo", [P, NHL, HID], BF16, kind="ExternalInput")[:]
    bqd = nc.dram_tensor("bq", [P, NHL], F32, kind="ExternalInput")[:]
    cpackd = nc.dram_tensor("cpack", [P, CPACK_W], BF16, kind="ExternalInput")[:]
    sel8d = nc.dram_tensor("sel8", [NHL, NHL * P], BF16, kind="ExternalInput")[:]
    outTd = nc.dram_tensor("outT", [HID, S], BF16, kind="ExternalOutput")[:]

    with tile.TileContext(nc) as tc, contextlib.ExitStack() as _les:
        if loop_iters is not None:
            _les.enter_context(tc.For_i(0, loop_iters, 1))
        with (
            tc.tile_pool(name="consts", bufs=1) as consts,
            tc.tile_pool(name="resident", bufs=1) as resident,
        ):
            cpack = consts.tile([P, CPACK_W], BF16)
            cos2_sb = cpack[:, 0:2 * S]
            sin2_sb = cpack[:, 2 * S:4 * S]
            tri_sb = cpack[:, 4 * S:4 * S + P]
            selo_sb = cpack[:, 4 * S + P:4 * S + P + NHL * P]
            bq_sb = consts.tile([P, NHL], F32)
            sel8_sb = consts.tile([NHL, NHL * P], BF16)

            latT = resident.tile([P, KT_L * S], BF16)
            qT = resident.tile([P, NHL * S], BF16)
            kT = resident.tile([P, NHL * S], BF16)
            v_sb = resident.tile([P, NHL * HL], BF16)
            ctxT = resident.tile([P, NHL * S], BF16)
            # phase-B weights, loaded during phase A
            wkg = resident.tile([P, KT_L * HL], BF16)
            wvg = resident.tile([P, KT_L * HL], BF16)

            pacc_cm = tc.tile_pool(name="pacc", bufs=8, space="PSUM")
            pacc = pacc_cm.__enter__()

            def rope_pair(rp, h, ps4, dst, bias, sin_eng,
                          add_eng=None, dma_eng=None):
                """RoPE for heads h, h+1 from 4 psum tiles [(j,ntc)]."""
                add_eng = add_eng or nc.vector
                dma_eng = dma_eng or nc.scalar
                raw = rp.tile([P, 2 * S], BF16, tag="raw", name="raw")
                sh = rp.tile([P, 2 * S], BF16, tag="sh", name="sh")
                for j in range(2):
                    for ntc in range(NQC):
                        seg = raw[:, (j * NQC + ntc) * QCW:
                                  (j * NQC + ntc + 1) * QCW]
                        if bias:
                            nc.scalar.add(seg, ps4[j * 2 + ntc],
                                          bq_sb[:, h + j:h + j + 1])
                        else:
                            nc.scalar.copy(seg, ps4[j * 2 + ntc])
                    # per-head shift: unblocks as soon as this head's two
                    # segment copies land (not the whole pair)
                    dma_eng.dma_start(sh[0:64, j * S:(j + 1) * S],
                                      raw[64:128, j * S:(j + 1) * S])
                    dma_eng.dma_start(sh[64:128, j * S:(j + 1) * S],
                                      raw[0:64, j * S:(j + 1) * S])
                out = dst[:, h * S:(h + 2) * S]
                nc.vector.tensor_mul(out, raw, cos2_sb)
                sin_eng.tensor_mul(sh, sh, sin2_sb)
                add_eng.tensor_add(out, out, sh)

            # ---------- phase A: QT (2 groups of 4 heads) + latT ----------
            with (
                tc.tile_pool(name="xp", bufs=1) as xp,
                tc.tile_pool(name="ws1", bufs=2) as ws1,
                tc.tile_pool(name="ropeA", bufs=2) as rpA,
            ):
                xT_sb = xp.tile([P, KT_H * S], BF16)
                wg0 = ws1.tile([P, KT_H * QCW], BF16, tag="w", name="wg")
                # x/wq interleaved in PE consumption order (kt ascending);
                # first pair small so the PE starts ~1us in
                nc.sync.dma_start(xT_sb[:, 0:S], xTd[:, 0:1, :])
                nc.sync.dma_start(wg0[:, 0:2 * QCW], wqd[:, 0:2, 0:QCW])
                nc.sync.dma_start(xT_sb[:, S:2 * S], xTd[:, 1:2, :])
                nc.sync.dma_start(wg0[:, 2 * QCW:4 * QCW],
                                  wqd[:, 2:4, 0:QCW])
                nc.sync.dma_start(xT_sb[:, 2 * S:4 * S], xTd[:, 2:4, :])
                nc.sync.dma_start(wg0[:, 4 * QCW:8 * QCW],
                                  wqd[:, 4:8, 0:QCW])
                nc.sync.dma_start(xT_sb[:, 4 * S:6 * S], xTd[:, 4:6, :])
                nc.sync.dma_start(wg0[:, 8 * QCW:12 * QCW],
                                  wqd[:, 8:12, 0:QCW])
                nc.sync.dma_start(xT_sb[:, 6 * S:8 * S], xTd[:, 6:8, :])
                nc.sync.dma_start(wg0[:, 12 * QCW:16 * QCW],
                                  wqd[:, 12:16, 0:QCW])
                nc.sync.dma_start(xT_sb[:, 8 * S:12 * S], xTd[:, 8:12, :])
                nc.sync.dma_start(xT_sb[:, 12 * S:16 * S], xTd[:, 12:16, :])
                # preload the exp table set while the PE is busy with
                # projections (first ACTIVATE otherwise pays ~2.7us in C)
                warm = ws1.tile([1, NHL], F32, tag="warm", name="warm")
                nc.scalar.activation(
                    out=warm, in_=bq_sb[0:1, :],
                    func=mybir.ActivationFunctionType.Exp, scale=1.0)

                for og in range(2):
                    if og == 0:
                        wg = wg0
                    else:
                        wg = ws1.tile([P, KT_H * QCW], BF16, tag="w",
                                      name="wg")
                        for hf in range(2):   # two 1MB halves
                            nc.sync.dma_start(
                                wg[:, hf * 8 * QCW:(hf + 1) * 8 * QCW],
                                wqd[:, hf * 8:(hf + 1) * 8,
                                    og * QCW:(og + 1) * QCW])
                    ps = [pacc.tile([P, QCW], F32, tag="acc", name="acc")
                          for _ in range(8)]
                    for kt in range(KT_H):
                        for oi in range(4):
                            for ntc in range(NQC):
                                nc.tensor.matmul(
                                    ps[oi * 2 + ntc],
                                    lhsT=wg[:, kt * QCW + oi * P:
                                            kt * QCW + (oi + 1) * P],
                                    rhs=xT_sb[:, kt * S + ntc * QCW:
                                              kt * S + (ntc + 1) * QCW],
                                    start=(kt == 0),
                                    stop=(kt == KT_H - 1),
                                )
                    if og == 0:
                        # consts behind the x/w stream on sync: the queue
                        # is busy until ~17us so these don't eat the HBM
                        # bandwidth the head refill needs, yet land well
                        # before their first uses
                        nc.sync.dma_start(bq_sb, bqd)
                        nc.sync.dma_start(cpack, cpackd)
                        nc.sync.dma_start(sel8_sb, sel8d)
                    for pr in range(2):
                        rope_pair(rpA, og * 4 + pr * 2,
                                  ps[pr * 4:pr * 4 + 4], qT, bias=True,
                                  sin_eng=nc.gpsimd)

                # latT group (4 out tiles x 2 chunks)
                wg = ws1.tile([P, KT_H * QCW], BF16, tag="w", name="wg")
                for hf in range(2):
                    nc.sync.dma_start(
                        wg[:, hf * 8 * QCW:(hf + 1) * 8 * QCW],
                        wdownd[:, hf * 8:(hf + 1) * 8, :])
                nc.sync.dma_start(wkg, wkupd)
                nc.sync.dma_start(wvg, wvupd)
                ps = [pacc.tile([P, QCW], F32, tag="acc", name="acc")
                      for _ in range(8)]
                for kt in range(KT_H):
                    for oi in range(4):
                        for ntc in range(NQC):
                            nc.tensor.matmul(
                                ps[oi * 2 + ntc],
                                lhsT=wg[:, kt * QCW + oi * P:
                                        kt * QCW + (oi + 1) * P],
                                rhs=xT_sb[:, kt * S + ntc * QCW:
                                          kt * S + (ntc + 1) * QCW],
                                start=(kt == 0),
                                stop=(kt == KT_H - 1),
                            )
                for oi in range(4):
                    for ntc in range(NQC):
                        dstap = latT[:, oi * S + ntc * QCW:
                                     oi * S + (ntc + 1) * QCW]
                        if ntc == 0:
                            nc.scalar.copy(dstap, ps[oi * 2 + ntc])
                        else:
                            nc.vector.tensor_copy(dstap, ps[oi * 2 + ntc])

            # ---------- phase B: KT (rope) then V ----------
            # wo on the SCALAR DMA queue: a 4MB transfer on sync would
            # head-of-line block phase-B's rope-shift DMAs (which stall
            # scalar/vector rope work and delay attention start)
            wop_cm = tc.tile_pool(name="wop", bufs=1)
            wop = wop_cm.__enter__()
            wo_sb = wop.tile([P, NHL * HID], BF16)
            nc.scalar.dma_start(wo_sb, wod)

            rpB_cm = tc.tile_pool(name="ropeB", bufs=2)
            rpB = rpB_cm.__enter__()

            for og in range(2):
                ps = [pacc.tile([P, QCW], F32, tag="acc", name="acc")
                      for _ in range(8)]
                for kt in range(KT_L):
                    for oi in range(4):
                        for ntc in range(NQC):
                            nc.tensor.matmul(
                                ps[oi * 2 + ntc],
                                lhsT=wkg[:, kt * HL + og * 4 * P + oi * P:
                                         kt * HL + og * 4 * P
                                         + (oi + 1) * P],
                                rhs=latT[:, kt * S + ntc * QCW:
                                         kt * S + (ntc + 1) * QCW],
                                start=(kt == 0),
                                stop=(kt == KT_L - 1),
                            )
                for pr in range(2):
                    sin = nc.vector if og == 0 else nc.gpsimd
                    rope_pair(rpB, og * 4 + pr * 2,
                              ps[pr * 4:pr * 4 + 4], kT, bias=False,
                              sin_eng=sin, add_eng=nc.vector,
                              dma_eng=nc.sync)

            # V up-proj in 4-bank groups so the last groups' copies drain
            # while later matmuls run and attention's PSUM banks free early
            for hlc in range(2):
                for g in range(2):
                    ps = [pacc.tile([P, QCW], F32, tag="acc", name="acc")
                          for _ in range(4)]
                    for kt in range(KT_L):
                        for s4 in range(4):
                            st = g * 4 + s4
                            nc.tensor.matmul(
                                ps[s4],
                                lhsT=latT[:, kt * S + st * P:
                                          kt * S + (st + 1) * P],
                                rhs=wvg[:, kt * HL + hlc * QCW:
                                        kt * HL + (hlc + 1) * QCW],
                                start=(kt == 0),
                                stop=(kt == KT_L - 1),
                            )
                    for s4 in range(4):
                        st = g * 4 + s4
                        dstap = v_sb[:, st * HL + hlc * QCW:
                                     st * HL + (hlc + 1) * QCW]
                        if s4 % 2 == 1:
                            nc.vector.tensor_copy(dstap, ps[s4])
                        else:
                            nc.scalar.copy(dstap, ps[s4])

            rpB_cm.__exit__(None, None, None)
            pacc_cm.__exit__(None, None, None)

            # ---------- phase C: attention + out-projection ----------
            with (
                tc.tile_pool(name="pctx", bufs=2, space="PSUM") as pctx,
                tc.tile_pool(name="psums", bufs=1, space="PSUM") as psums,
                tc.tile_pool(name="exla", bufs=3) as exla,
                tc.tile_pool(name="exlb", bufs=3) as exlb,
                tc.tile_pool(name="small", bufs=2) as small,
                tc.tile_pool(name="outsb", bufs=6) as outsb,
            ):
                fill = []

                def fl_pop(n, reserve=0):
                    for _ in range(n):
                        if len(fill) > reserve:
                            fill.pop(0)()

                def add_outproj(qc, ot, pbc, store_eng):
                    st_ = {}

                    def mk_mm(kt):
                        def go():
                            if kt == 0:
                                st_["po"] = pbc.tile([P, QCW], F32,
                                                     tag="bcpo", name="po")
                            nc.tensor.matmul(
                                st_["po"],
                                lhsT=wo_sb[:, kt * HID + ot * P:
                                           kt * HID + (ot + 1) * P],
                                rhs=ctxT[:, kt * S + qc * QCW:
                                         kt * S + (qc + 1) * QCW],
                                start=(kt == 0),
                                stop=(kt == NHL - 1),
                            )
                        return go

                    def fin():
                        ob = outsb.tile([P, QCW], BF16, tag="osb", name="ob")
                        nc.vector.tensor_copy(ob, st_["po"])
                        store_eng.dma_start(
                            outTd[ot * P:(ot + 1) * P,
                                  qc * QCW:(qc + 1) * QCW], ob)

                    for kt in range(NHL):
                        fill.append(mk_mm(kt))
                    fill.append(fin)

                def att_unit(h, qc, sums_ps, sctr, stot, exp_pool, psc,
                             tri_engs, ahead=1):
                    """Generator: one attention head, yields per kt step."""
                    nkt = 4 * qc + 4
                    ctx = pctx.tile([P, QCW], F32, tag="ctx", name="ctx")

                    def geom(kt):
                        off = kt - 4 * qc
                        if off < 0:
                            return 0, QCW, False
                        return 128 * off, QCW - 128 * off, True

                    def emit_sc(kt):
                        lo, w, diag = geom(kt)
                        sc = psc.tile([P, QCW], F32, tag="sc", name="sc")
                        nc.tensor.matmul(
                            sc[:, :w],
                            lhsT=kT[:, h * S + kt * P:h * S + (kt + 1) * P],
                            rhs=qT[:, h * S + qc * QCW + lo:
                                   h * S + qc * QCW + lo + w],
                            start=True, stop=True,
                        )
                        ex = exp_pool.tile([P, QCW], BF16, tag="ex",
                                           name="ex")
                        nc.scalar.activation(
                            out=ex[:, :w], in_=sc[:, :w],
                            func=mybir.ActivationFunctionType.Exp,
                            scale=SC_SCALE,
                        )
                        if diag:
                            eng = tri_engs[kt % 2]
                            eng.tensor_mul(ex[:, 0:P], ex[:, 0:P], tri_sb)
                        return ex

                    def emit_pv(kt, ex):
                        lo, w, _ = geom(kt)
                        nc.tensor.matmul(
                            ctx[:, lo:lo + w],
                            lhsT=v_sb[:, kt * HL + h * P:
                                      kt * HL + (h + 1) * P],
                            rhs=ex[:, :w],
                            start=(kt == 0),
                            stop=(kt == nkt - 1),
                        )
                        i = sctr[0]
                        sctr[0] += 1
                        nc.tensor.matmul(
                            sums_ps[:, lo:lo + w],
                            lhsT=selo_sb[:, h * P:(h + 1) * P],
                            rhs=ex[:, :w],
                            start=(i == 0),
                            stop=(i == stot - 1),
                        )

                    exs = {k: emit_sc(k) for k in range(min(ahead, nkt))}
                    for kt in range(nkt):
                        if kt + ahead < nkt:
                            exs[kt + ahead] = emit_sc(kt + ahead)
                        emit_pv(kt, exs.pop(kt))
                        yield
                    ctx_dst = ctxT[:, h * S + qc * QCW:
                                   h * S + (qc + 1) * QCW]
                    if qc == 0:
                        if h % 2 == 0:
                            nc.scalar.copy(ctx_dst, ctx)
                        else:
                            nc.vector.tensor_copy(ctx_dst, ctx)
                    else:
                        nc.vector.tensor_copy(ctx_dst, ctx)

                def drive(gens, live_n, pops, reserve=0):
                    """Rolling scheduler: keep live_n generators in flight;
                    pop `pops` fill closures after each yield, keeping
                    `reserve` closures back for the pass boundary."""
                    q = list(gens)
                    live = []
                    while q or live:
                        while q and len(live) < live_n:
                            live.append(q.pop(0))
                        for g in list(live):
                            try:
                                next(g)
                            except StopIteration:
                                live.remove(g)
                            fl_pop(pops, reserve)

                def finish8(qc, sums_ps, pbc_ref):
                    """Copy sums rows out, reciprocal, return norm closure."""
                    srow = small.tile([NHL, QCW], F32, tag="srow",
                                      name="srow")
                    nc.vector.tensor_copy(srow, sums_ps[0:NHL, :])
                    rec = small.tile([NHL, QCW], BF16, tag="rec", name="rec")
                    with nc.allow_low_precision(reason="bf16 softmax "
                                                "denominator (gate 2e-2)"):
                        nc.vector.reciprocal(out=rec, in_=srow)

                    def go():
                        for h in range(NHL):
                            bc = pbc_ref[0].tile([P, QCW], F32, tag="bcpo",
                                                 name="bc")
                            nc.tensor.matmul(
                                bc,
                                lhsT=sel8_sb[:, h * P:(h + 1) * P],
                                rhs=rec,
                                start=True, stop=True,
                            )
                            sl = ctxT[:, h * S + qc * QCW:
                                      h * S + (qc + 1) * QCW]
                            nc.vector.tensor_mul(sl, sl, bc)
                    return go

                pbc_ref = [None]

                # ---- pass Q0: all 8 heads, q-chunk 0 (two heads in flight)
                pscq0_cm = tc.tile_pool(name="pscq0", bufs=4, space="PSUM")
                pscq0 = pscq0_cm.__enter__()
                sums_q0 = psums.tile([P, QCW], F32, tag="sums", name="sums")
                c0 = [0]
                gens = [
                    att_unit(h, 0, sums_q0, c0, NHL * 4,
                             exla if h % 2 == 0 else exlb, pscq0,
                             (nc.vector, nc.gpsimd) if h % 2 == 0
                             else (nc.gpsimd, nc.vector))
                    for h in range(NHL)
                ]
                drive(gens, live_n=2, pops=1)
                norm_q0 = finish8(0, sums_q0, pbc_ref)
                pscq0_cm.__exit__(None, None, None)

                # ---- pass Q1: all 8 heads q-chunk 1; fill = qc0 norms +
                # qc0 out-projection
                pbc_cm = tc.tile_pool(name="pbc", bufs=2, space="PSUM")
                pbc = pbc_cm.__enter__()
                pscq1_cm = tc.tile_pool(name="pscq1", bufs=3, space="PSUM")
                pscq1 = pscq1_cm.__enter__()
                pbc_ref[0] = pbc
                fill.append(norm_q0)
                for ot in range(HID // P):
                    add_outproj(0, ot, pbc, nc.sync)
                sums_q1 = psums.tile([P, QCW], F32, tag="sums",
                                     name="sums")
                c1 = [0]
                gens = [
                    att_unit(h, 1, sums_q1, c1, NHL * 8,
                             exla if h % 2 == 0 else exlb, pscq1,
                             (nc.gpsimd, nc.vector), ahead=2)
                    for h in range(NHL)
                ]
                drive(gens, live_n=1, pops=2, reserve=18)
                # issue sums copy + reciprocal now; drain leftover qc0
                # out-proj to cover the reciprocal latency
                norm_q1 = finish8(1, sums_q1, pbc_ref)
                while fill:
                    fill.pop(0)()
                norm_q1()
                # attention pools done: swap psc banks for a deeper
                # out-proj pipeline (avoids pbc/outsb ring WAR stalls at
                # the tail)
                pscq1_cm.__exit__(None, None, None)
                with tc.tile_pool(name="pout", bufs=3,
                                  space="PSUM") as pout:
                    for ot in range(HID // P):
                        add_outproj(1, ot, pout,
                                    nc.scalar if ot % 2 else nc.sync)
                    while fill:
                        fill.pop(0)()
                pbc_cm.__exit__(None, None, None)

            wop_cm.__exit__(None, None, None)
    nc.compile()
    return nc


# ---------------- host side ----------------

def _host_consts():
    inv_freq = 1.0 / (10000.0 ** (np.arange(0, HD, 2, dtype=np.float64) / HD))
    t = np.arange(S, dtype=np.float64)
    freqs = t[:, None] * inv_freq[None, :]            # [S, 64]
    emb = np.concatenate([freqs, freqs], axis=-1)     # [S, 128]
    cosT = np.cos(emb).T.astype(np.float32)           # [128, S]
    sinT = np.sin(emb).T.astype(np.float32)
    sinTe = sinT.copy()
    sinTe[:64] *= -1.0                                # rotate_half sign folded
    cos2 = np.broadcast_to(cosT[:, None, :], (P, 2, S)).reshape(P, 2 * S)
    sin2 = np.broadcast_to(sinTe[:, None, :], (P, 2, S)).reshape(P, 2 * S)

    ii = np.arange(P)[:, None]
    tri = (np.arange(P)[None, :] - ii >= 0).astype(np.float32)  # [128,128]

    selones = np.zeros((P, NHL * P), dtype=np.float32)
    for h in range(NHL):
        selones[:, h * P + h] = 1.0
    cpack = np.ascontiguousarray(
        np.concatenate([cos2, sin2, tri, selones], axis=1)).astype(NPBF)

    sel8 = np.zeros((NHL, NHL * P), dtype=NPBF)
    for h in range(NHL):
        sel8[h, h * P:(h + 1) * P] = 1.0
    return cpack, sel8


_CACHE = {}


def _get_built():
    if "nc" not in _CACHE:
        _CACHE["nc"] = build_bass()
        _CACHE["consts"] = _host_consts()
    return _CACHE["nc"], _CACHE["consts"]


def _swz(a, n_kt):
    """[n_kt*128, W] -> [128, n_kt, W] (partition-major swizzle), bf16."""
    w = a.shape[1]
    return np.ascontiguousarray(
        a.reshape(n_kt, P, w).transpose(1, 0, 2)).astype(NPBF)


def make_in_maps(x, Wq, bq, Wdown, Wk_up, Wv_up, Wo):
    cpack, sel8 = _get_built()[1]
    in_maps = []
    for c in range(N_CORES):
        b, hg = c // 2, c % 2
        sl = slice(hg * HL, (hg + 1) * HL)
        in_maps.append({
            "xT": _swz(np.ascontiguousarray(x[b].T), KT_H),
            "wq": _swz(Wq[:, sl], KT_H),
            "wdown": _swz(Wdown, KT_H),
            "wkup": _swz(Wk_up[:, sl], KT_L),
            "wvup": _swz(Wv_up[:, sl], KT_L),
            "wo": _swz(Wo[sl, :], NHL),
            "bq": np.ascontiguousarray(
                bq[sl].reshape(NHL, P).T).astype(np.float32),
            "cpack": cpack,
            "sel8": sel8,
        })
    return in_maps


def gather_out(results, bo):
    out = np.empty((B, S, HID), dtype=np.float32)
    for b in range(B):
        acc = (results[2 * b]["outT"].astype(np.float32)
               + results[2 * b + 1]["outT"].astype(np.float32))  # [HID, S]
        out[b] = acc.T + bo[None, :]
    return out


def kernel(x, Wq, bq, Wdown, Wk_up, Wv_up, Wo, bo):
    x = np.asarray(x, dtype=np.float32)
    Wq = np.asarray(Wq, dtype=np.float32)
    bq = np.asarray(bq, dtype=np.float32)
    Wdown = np.asarray(Wdown, dtype=np.float32)
    Wk_up = np.asarray(Wk_up, dtype=np.float32)
    Wv_up = np.asarray(Wv_up, dtype=np.float32)
    Wo = np.asarray(Wo, dtype=np.float32)
    bo = np.asarray(bo, dtype=np.float32)

    nc, _ = _get_built()
    in_maps = make_in_maps(x, Wq, bq, Wdown, Wk_up, Wv_up, Wo)
    res = run_bass_kernel_spmd(nc, in_maps, core_ids=list(range(N_CORES)))
    return gather_out(res.results, bo)
